# revision 24
# baseline (speedup 1.0000x reference)
"""GRU decoder Trainium2 kernel (data-parallel over batch, 8 cores).

Reference (per step t, PyTorch nn.GRU gate order r,z,n):
    gi = x @ w_ih.T + b_ih ; gh = h @ w_hh.T + b_hh
    r = sig(i_r + h_r); z = sig(i_z + h_z); n = tanh(i_n + r * h_n)
    h' = (1-z)*n + z*h ; y = h' @ w_fc.T + b_fc ; x <- y
Shapes: H=1024, O=768, B=256, T=256.  Each core handles 32 batch rows.

Structure (v16 - quarter-streamed combine + h0-first DVE FIFO):
  * x_t = y_{t-1} folds into the hidden-side matmuls, so every recurrent
    matmul contracts over H=1024: regions r, hn (= h_n), z, in (= i_n).
  * The state lives ONLY as hsb = h'^T (bf16, PE lhsT layout).
  * rt = sig(r)*hn (bf16) is ACCUMULATED INTO the gI PSUM by per-quadrant
    diagonal-identity matmuls, so n = tanh(gI) reads PSUM directly - no
    DVE add / extra sem hop on the critical tail.  bf16 rt keeps the
    inject to one cheap round (fp32 would lower to 2 LOW_HIGH passes).
  * The n-side tail is HALVED: tanh / n^T-transpose / vT / hsb' run per
    128-col half into per-bank PSUM tiles, and the next step's r|hn
    matmuls are issued even-chunks-first (even chunks only read the h0
    half of the fresh state), so they start one half earlier.
  * y_t writes its own PSUM bank (no DVE-read-vs-PE-write bank stall on
    the combine) and is split around the n^T transposes to fill PE gaps.
  * Biases seed PSUM via K=1 ones-row matmuls.  One start=True per bank.
  * Step-0 gates come from the host; b_fc is added on the host.
"""

import numpy as np
import ml_dtypes

import concourse.bass as bass
import concourse.bacc as bacc
import concourse.tile as tile
from concourse import mybir
from concourse.bass_utils import run_bass_kernel_spmd

H = 1024
O = 768
B = 256
T = 256
NCORES = 8
BC = B // NCORES  # 32 batch rows per core

KH = H // 128  # 8 contraction chunks
NGATE = 4      # regions r, hn, z, in (issue order)
YW = O // 4    # 192 y cols per quadrant

F32 = mybir.dt.float32
BF16 = mybir.dt.bfloat16
AF = mybir.ActivationFunctionType
ALU = mybir.AluOpType

_COMPILED = None

# bf16 const layout: WG | WF | ONES | BIAS | IB32 | IB128
WG_N = NGATE * KH * 4 * 256   # 32768
WF_N = KH * 4 * YW            # 6144
NB = WG_N + WF_N + 32 + 4096 + 32 + 128  # 43200
# f32 const layout: G0 (r|hn|z|in) | H0T | IDT | ONESF
NF = NGATE * 256 + 256 + 128 + 256  # 1664

KEVEN = (0, 1, 2, 3)
KODD = (4, 5, 6, 7)


def _hslice(hsb, k):
    """lhsT chunk k from the block-transposed state tile: cols 32k..32k+32.
    Partition 32i+a of chunk k holds h-feature 256i + 128*(k//4) +
    32*(k%4) + a; the weight packing uses the same row permutation."""
    return hsb[:, 32 * k : 32 * k + 32]


def _build_nc():
    nc = bacc.Bacc("TRN2", target_bir_lowering=False, debug=False, num_devices=NCORES)

    cb = nc.declare_dram_parameter("CB", [128, NB], BF16, isOutput=False)
    cf = nc.declare_dram_parameter("CF", [128, NF], F32, isOutput=False)
    o = nc.declare_dram_parameter("O", [T, 128, YW], F32, isOutput=True)

    with tile.TileContext(nc) as tc:
        with (
            tc.tile_pool(name="wpool", bufs=1) as wpool,
            tc.tile_pool(name="state", bufs=2) as spool,
            tc.tile_pool(name="act", bufs=2) as apool,
            tc.tile_pool(name="gps", bufs=1, space="PSUM") as gpool,
        ):
            CB = wpool.tile([128, NB], BF16, tag="CB")
            CF = wpool.tile([128, NF], F32, tag="CF")
            nc.sync.dma_start(CB[:], cb[:])
            nc.sync.dma_start(CF[:], cf[:])
            WG = CB[:, 0:WG_N]
            WF = CB[:, WG_N : WG_N + WF_N]
            ONES = CB[0:1, WG_N + WF_N : WG_N + WF_N + 32]
            BIAS = CB[0:1, WG_N + WF_N + 32 : WG_N + WF_N + 32 + 4096]
            IB32 = CB[:, WG_N + WF_N + 32 + 4096 : WG_N + WF_N + 32 + 4128]  # 4x I32
            IB128 = CB[:, WG_N + WF_N + 32 + 4128 : NB]  # bf16 I128
            G0 = CF[:, 0 : NGATE * 256]
            H0T = CF[:, NGATE * 256 : NGATE * 256 + 256]
            IDT = CF[:, NGATE * 256 + 256 : NGATE * 256 + 384]
            ONESF = CF[:, NGATE * 256 + 384 : NF]  # all-ones f32 [128,256]

            # PSUM: 8 banks exactly: gA x2 | gZ | gI | tpZ | tpN0 | tpN1 | tpY
            def mk_gA():
                gA = gpool.tile([128, 512], F32, tag="gA", name="gA", bufs=2)
                return gA

            def mk(tag, n, bufs=1):
                # bank-padded (512 f32) so no two PSUM tiles share a bank;
                # hand back a view of the first n cols
                full = gpool.tile([128, 512], F32, tag=tag, name=tag, bufs=bufs)
                return full[:, 0:n]

            def emit_biasA(gA):
                for j in range(4):
                    nc.tensor.matmul(
                        gA[32 * j : 32 * j + 32, :],
                        ONES[:, 0:32],
                        BIAS[:, 512 * j : 512 * j + 512],
                        start=True, stop=False, tile_position=(0, 32 * j),
                    )

            def emit_biasZI(gt, gi):
                for j in range(4):
                    bofs = 1024 * gi + 256 * j
                    nc.tensor.matmul(
                        gt[32 * j : 32 * j + 32, :],
                        ONES[:, 0:32],
                        BIAS[:, bofs : bofs + 256],
                        start=True, stop=False, tile_position=(0, 32 * j),
                    )

            def mk_gates():
                return mk_gA(), mk("gZ", 256, bufs=2), mk("gI", 256, bufs=2)

            def emit_A(hsb, gA):
                # r|hn pair as single N=512 matmuls; even chunks first so
                # the round 0 only waits on the h0 half of the new state.
                for i, k in enumerate(KEVEN + KODD):
                    lhsT = _hslice(hsb, k)
                    for j in range(4):
                        wofs = (k * 4 + j) * 512
                        nc.tensor.matmul(
                            gA[32 * j : 32 * j + 32, :],
                            lhsT,
                            WG[:, wofs : wofs + 512],
                            start=False,
                            stop=(i == KH - 1),
                            tile_position=(0, 32 * j),
                        )

            def emit_ZI(hsb, gt, gi, stop_last):
                for k in range(KH):
                    lhsT = _hslice(hsb, k)
                    for j in range(4):
                        wofs = 16384 + ((gi * KH + k) * 4 + j) * 256
                        nc.tensor.matmul(
                            gt[32 * j : 32 * j + 32, :],
                            lhsT,
                            WG[:, wofs : wofs + 256],
                            start=False,
                            stop=(stop_last and k == KH - 1),
                            tile_position=(0, 32 * j),
                        )

            def emit_inject(gI, rtb):
                # gI += I32^T @ rt per quadrant (bf16, diagonal PE tiles)
                for j in range(4):
                    nc.tensor.matmul(
                        gI[32 * j : 32 * j + 32, :],
                        IB32[32 * j : 32 * j + 32, 0:32],
                        rtb[32 * j : 32 * j + 32, :],
                        start=False,
                        stop=True,
                        tile_position=(32 * j, 32 * j),
                    )

            def emit_y(hsb_t, tpY, kset):
                for k in kset:
                    lhsT = _hslice(hsb_t, k)
                    for j in range(4):
                        wofs = (k * 4 + j) * YW
                        nc.tensor.matmul(
                            tpY[32 * j : 32 * j + 32, 0:YW],
                            lhsT,
                            WF[:, wofs : wofs + YW],
                            start=(k == 0),
                            stop=(k == KH - 1),
                            tile_position=(0, 32 * j),
                        )

            def chain_zside(zs, tpZ, hsb_prev):
                """p^T = z^T * h^T per half on DVE (early, off the tail)."""
                pts = []
                for hh in range(2):
                    sl = slice(128 * hh, 128 * hh + 128)
                    pT = apool.tile([128, 128], F32, tag=f"pT{hh}", name=f"pT{hh}")
                    nc.vector.tensor_tensor(pT[:], tpZ[:, sl], hsb_prev[:, sl], ALU.mult)
                    pts.append(pT)
                return pts, (tpZ, hsb_prev)

            def chain_nquarter(n_h, pts, pz, hsb2, hh):
                """per 64-col quarter: bt(n) ; q = (z^T-1)*n^T ;
                hsb'[quarter] = pT - q  (DVE) - each quarter unlocks two
                A-rounds of the next step as soon as it lands."""
                tpZ, _ = pz
                tpN = apool.tile([128, 128], BF16, tag=f"tpN{hh}b", name=f"tpN{hh}b")
                for qq in range(2):
                    sln = slice(64 * qq, 64 * qq + 64)
                    glob = slice(128 * hh + 64 * qq, 128 * hh + 64 * qq + 64)
                    nc.vector.transpose(tpN[:, sln], n_h[:, sln])
                    q = apool.tile([128, 64], F32, tag=f"q{hh}{qq}", name=f"q{hh}{qq}")
                    nc.vector.scalar_tensor_tensor(
                        q[:], tpZ[:, glob], 1.0, tpN[:, sln], ALU.subtract, ALU.mult
                    )
                    nc.vector.tensor_tensor(hsb2[:, glob], pts[hh][:, sln], q[:], ALU.subtract)

            # ---- step 0: gates computed host-side (biases included) ----
            rs0 = apool.tile([128, 256], F32, tag="rs")
            nc.scalar.activation(rs0[:], G0[:, 0:256], AF.Sigmoid)
            zs0 = apool.tile([128, 256], BF16, tag="zs")
            nc.scalar.activation(zs0[:], G0[:, 512:768], AF.Sigmoid)
            rt0 = apool.tile([128, 256], F32, tag="rt0")
            nc.vector.tensor_tensor(rt0[:], rs0[:], G0[:, 256:512], ALU.mult)
            ns0 = apool.tile([128, 256], F32, tag="ns0")
            nc.vector.tensor_tensor(ns0[:], rt0[:], G0[:, 768:1024], ALU.add)
            n0 = apool.tile([128, 256], BF16, tag="n0")
            nc.scalar.activation(n0[:], ns0[:], AF.Tanh)
            tpZ = apool.tile([128, 256], BF16, tag="tpZb")
            nc.vector.transpose(tpZ[:, 0:128], zs0[:, 0:128])
            nc.vector.transpose(tpZ[:, 128:256], zs0[:, 128:256])
            pts, pz = chain_zside(zs0, tpZ, H0T)
            hsb = spool.tile([128, 256], BF16, tag="hsb")
            chain_nquarter(n0[:, 0:128], pts, pz, hsb, 0)
            chain_nquarter(n0[:, 128:256], pts, pz, hsb, 1)

            # gates for step 1: bias seeded up front
            pend = mk_gates()
            emit_biasA(pend[0])
            emit_biasZI(pend[1], 2)
            emit_biasZI(pend[2], 3)

            for t in range(T):
                last = t == T - 1
                tpY = mk("tpY", YW)
                if not last:
                    gA, gZ, gI = pend
                    emit_A(hsb, gA)
                    emit_ZI(hsb, gZ, 0, stop_last=True)

                    rs = apool.tile([128, 256], F32, tag="rs")
                    nc.scalar.activation(rs[:], gA[:, 0:256], AF.Sigmoid)
                    zs = apool.tile([128, 256], BF16, tag="zs")
                    nc.scalar.activation(zs[:], gZ[:], AF.Sigmoid)
                    rtb = apool.tile([128, 256], BF16, tag="rtb")
                    nc.vector.tensor_tensor(rtb[:], rs[:], gA[:, 256:512], ALU.mult)

                    emit_ZI(hsb, gI, 1, stop_last=False)
                    emit_inject(gI, rtb)

                    tpZ = apool.tile([128, 256], BF16, tag="tpZb")
                    nc.vector.transpose(tpZ[:, 0:128], zs[:, 0:128])
                    nc.vector.transpose(tpZ[:, 128:256], zs[:, 128:256])

                    n_h0 = apool.tile([128, 128], BF16, tag="n_h0")
                    nc.scalar.activation(n_h0[:], gI[:, 0:128], AF.Tanh)
                    n_h1 = apool.tile([128, 128], BF16, tag="n_h1")
                    nc.scalar.activation(n_h1[:], gI[:, 128:256], AF.Tanh)

                    pts, pz = chain_zside(zs, tpZ, hsb[:])

                    # PE: next-step bias rides the streams, then the whole
                    # y block fills the chain window.
                    if t < T - 2:
                        pend = mk_gates()
                        emit_biasZI(pend[1], 2)
                        emit_biasZI(pend[2], 3)
                    emit_y(hsb, tpY, KEVEN + KODD)
                    if t < T - 2:
                        emit_biasA(pend[0])

                    # DVE FIFO: the whole h0 chain precedes the h1 ops,
                    # so it is not gated by tanh_h1; quarters stream out.
                    hsb2 = spool.tile([128, 256], BF16, tag="hsb")
                    chain_nquarter(n_h0, pts, pz, hsb2, 0)
                    chain_nquarter(n_h1, pts, pz, hsb2, 1)
                    hsb = hsb2
                else:
                    emit_y(hsb, tpY, KEVEN + KODD)

                ys = apool.tile([128, YW], F32, tag="ys")
                nc.scalar.copy(ys[:], tpY[:, 0:YW])
                nc.sync.dma_start(o[t], ys[:])

    nc.compile()
    return nc


def _pack_bat(M):
    """[32, 4*W] -> [128, W]: row 32j+b holds M[b, W*j : W*j+W]."""
    w = M.shape[1] // 4
    return np.ascontiguousarray(
        M.reshape(BC, 4, w).transpose(1, 0, 2).reshape(128, w)
    )


def _prep_shared(w_ih, w_hh, b_ih, b_hh, w_fc, b_fc):
    wihT = w_ih.T.astype(np.float64)  # [768, 3072]
    whhT = w_hh.T.astype(np.float64)  # [1024, 3072]
    wfcT = w_fc.T.astype(np.float64)  # [1024, 768]
    fold = wfcT @ wihT                # [1024, 3072]
    Wr = fold[:, 0:H] + whhT[:, 0:H]
    Wz = fold[:, H : 2 * H] + whhT[:, H : 2 * H]
    Win = fold[:, 2 * H : 3 * H]
    Whn = whhT[:, 2 * H : 3 * H]

    bfold = b_fc.astype(np.float64) @ wihT  # [3072]
    br = bfold[0:H] + b_ih[0:H] + b_hh[0:H]
    bz = bfold[H : 2 * H] + b_ih[H : 2 * H] + b_hh[H : 2 * H]
    bin_ = bfold[2 * H :] + b_ih[2 * H :]
    bhn = b_hh[2 * H :].astype(np.float64)

    # chunk row permutation matching the DVE block-transposed state:
    # chunk c, partition p=32i+a  ->  h-feature 256i + 128*(c//4) + 32*(c%4) + a
    p = np.arange(128)
    cidx = [256 * (p // 32) + 128 * (c // 4) + 32 * (c % 4) + (p % 32) for c in range(KH)]

    blocks = []
    # r|hn interleaved per (k,j) for N=512 pair matmuls
    for k in range(KH):
        for j in range(4):
            blocks.append(Wr[cidx[k]][:, 256 * j : 256 * j + 256])
            blocks.append(Whn[cidx[k]][:, 256 * j : 256 * j + 256])
    # then z, in blocks (N=256)
    for G in (Wz, Win):
        for k in range(KH):
            for j in range(4):
                blocks.append(G[cidx[k]][:, 256 * j : 256 * j + 256])
    WGp = np.concatenate(blocks, axis=1).astype(ml_dtypes.bfloat16)  # [128, 32768]

    yblocks = []
    for k in range(KH):
        for j in range(4):
            yblocks.append(wfcT[cidx[k]][:, YW * j : YW * j + YW])
    WFp = np.concatenate(yblocks, axis=1).astype(ml_dtypes.bfloat16)  # [128, 6144]

    ones_col = np.zeros((128, 32), ml_dtypes.bfloat16)
    ones_col[0, :] = 1
    # bias layout: j-paired [br_j | bhn_j] (4x512) then bz (1024), bin (1024)
    bias_row = np.empty(4096, np.float64)
    for j in range(4):
        bias_row[512 * j : 512 * j + 256] = br[256 * j : 256 * j + 256]
        bias_row[512 * j + 256 : 512 * j + 512] = bhn[256 * j : 256 * j + 256]
    bias_row[2048:3072] = bz
    bias_row[3072:4096] = bin_
    bias_col = np.zeros((128, 4096), ml_dtypes.bfloat16)
    bias_col[0, :] = bias_row.astype(ml_dtypes.bfloat16)

    ib32 = np.tile(np.eye(32), (4, 1)).astype(ml_dtypes.bfloat16)  # [128, 32]
    ib128 = np.eye(128).astype(ml_dtypes.bfloat16)  # [128, 128]

    CBp = np.concatenate([WGp, WFp, ones_col, bias_col, ib32, ib128], axis=1)  # [128, NB]
    assert CBp.shape[1] == NB
    IDT = np.eye(128, dtype=np.float32)
    return CBp, IDT


def _build_in_maps(inputs):
    src = np.asarray(inputs["src"], np.float32)
    hidden = np.asarray(inputs["hidden"], np.float32)
    w_ih = np.asarray(inputs["w_ih"], np.float32)
    w_hh = np.asarray(inputs["w_hh"], np.float32)
    b_ih = np.asarray(inputs["b_ih"], np.float32)
    b_hh = np.asarray(inputs["b_hh"], np.float32)
    w_fc = np.asarray(inputs["w_fc"], np.float32)
    b_fc = np.asarray(inputs["b_fc"], np.float32)

    CBp, IDT = _prep_shared(w_ih, w_hh, b_ih, b_hh, w_fc, b_fc)

    # step-0 gates on host (f64): from x0=src[0], h0=hidden[0]
    x0 = src[0].astype(np.float64)
    h0 = hidden[0].astype(np.float64)
    gi0 = x0 @ w_ih.T.astype(np.float64) + b_ih.astype(np.float64)
    gh0 = h0 @ w_hh.T.astype(np.float64) + b_hh.astype(np.float64)
    g0r = gi0[:, 0:H] + gh0[:, 0:H]
    g0z = gi0[:, H : 2 * H] + gh0[:, H : 2 * H]
    g0in = gi0[:, 2 * H :]
    g0hn = gh0[:, 2 * H :]

    in_maps = []
    for c in range(NCORES):
        sl = slice(BC * c, BC * (c + 1))
        G0 = np.concatenate(
            [
                _pack_bat(g0r[sl]),
                _pack_bat(g0hn[sl]),
                _pack_bat(g0z[sl]),
                _pack_bat(g0in[sl]),
            ],
            axis=1,
        )  # [128, 1024] in region order r|hn|z|in
        HP0 = _pack_bat(h0[sl])  # [128, 256]
        # block-transposed state layout: per half, 32x32 blocks transposed
        # in place (matches nc.vector.transpose semantics)
        def blockT(X):
            return np.ascontiguousarray(
                X.reshape(4, 32, 4, 32).transpose(0, 3, 2, 1).reshape(128, 128)
            )
        H0T = np.concatenate(
            [blockT(HP0[:, 0:128]), blockT(HP0[:, 128:256])], axis=1
        )
        CFp = np.concatenate([G0, H0T, IDT, np.ones((128, 256), np.float32)], axis=1).astype(np.float32)
        assert CFp.shape[1] == NF
        in_maps.append(dict(CB=CBp, CF=CFp))
    return in_maps


def kernel(src, tgt, hidden, w_ih, w_hh, b_ih, b_hh, w_fc, b_fc, **_kw):
    global _COMPILED
    b_fc = np.asarray(b_fc, np.float32)

    if _COMPILED is None:
        _COMPILED = _build_nc()
    nc = _COMPILED

    in_maps = _build_in_maps(
        dict(src=src, hidden=hidden, w_ih=w_ih, w_hh=w_hh, b_ih=b_ih,
             b_hh=b_hh, w_fc=w_fc, b_fc=b_fc)
    )

    res = run_bass_kernel_spmd(nc, in_maps, list(range(NCORES)))

    out = np.empty((T, B, O), np.float32)
    for c in range(NCORES):
        sl = slice(BC * c, BC * (c + 1))
        oc = np.asarray(res.results[c]["O"])  # [T, 128, 192]
        out[:, sl, :] = (
            oc.reshape(T, 4, BC, YW).transpose(0, 2, 1, 3).reshape(T, BC, O)
        )
    out += b_fc[None, None, :]
    return out


# revision 25
# speedup vs baseline: 1.2121x; 1.2121x over previous
"""GRU decoder Trainium2 kernel (data-parallel over batch, 8 cores).

Reference (per step t, PyTorch nn.GRU gate order r,z,n):
    gi = x @ w_ih.T + b_ih ; gh = h @ w_hh.T + b_hh
    r = sig(i_r + h_r); z = sig(i_z + h_z); n = tanh(i_n + r * h_n)
    h' = (1-z)*n + z*h ; y = h' @ w_fc.T + b_fc ; x <- y
Shapes: H=1024, O=768, B=256, T=256.  Each core handles 32 batch rows.

Structure (v19 - v16 + single full-array inject):
  * x_t = y_{t-1} folds into the hidden-side matmuls, so every recurrent
    matmul contracts over H=1024: regions r, hn (= h_n), z, in (= i_n).
  * The state lives ONLY as hsb = h'^T (bf16, PE lhsT layout).
  * rt = sig(r)*hn (bf16) is ACCUMULATED INTO the gI PSUM by per-quadrant
    diagonal-identity matmuls, so n = tanh(gI) reads PSUM directly - no
    DVE add / extra sem hop on the critical tail.  bf16 rt keeps the
    inject to one cheap round (fp32 would lower to 2 LOW_HIGH passes).
  * The n-side tail is HALVED: tanh / n^T-transpose / vT / hsb' run per
    128-col half into per-bank PSUM tiles, and the next step's r|hn
    matmuls are issued even-chunks-first (even chunks only read the h0
    half of the fresh state), so they start one half earlier.
  * y_t writes its own PSUM bank (no DVE-read-vs-PE-write bank stall on
    the combine) and is split around the n^T transposes to fill PE gaps.
  * Biases seed PSUM via K=1 ones-row matmuls.  One start=True per bank.
  * Step-0 gates come from the host; b_fc is added on the host.
"""

import numpy as np
import ml_dtypes

import concourse.bass as bass
import concourse.bacc as bacc
import concourse.tile as tile
from concourse import mybir
from concourse.bass_utils import run_bass_kernel_spmd

H = 1024
O = 768
B = 256
T = 256
NCORES = 8
BC = B // NCORES  # 32 batch rows per core

KH = H // 128  # 8 contraction chunks
NGATE = 4      # regions r, hn, z, in (issue order)
YW = O // 4    # 192 y cols per quadrant

F32 = mybir.dt.float32
BF16 = mybir.dt.bfloat16
AF = mybir.ActivationFunctionType
ALU = mybir.AluOpType

_COMPILED = None

# bf16 const layout: WG | WF | ONES | BIAS | IB32 | IB128
WG_N = NGATE * KH * 4 * 256   # 32768
WF_N = KH * 4 * YW            # 6144
NB = WG_N + WF_N + 32 + 4096 + 32 + 128  # 43200
# f32 const layout: G0 (r|hn|z|in) | H0T | IDT | ONESF
NF = NGATE * 256 + 256 + 128 + 256  # 1664

KEVEN = (0, 1, 2, 3)
KODD = (4, 5, 6, 7)


def _hslice(hsb, k):
    """lhsT chunk k from the block-transposed state tile: cols 32k..32k+32.
    Partition 32i+a of chunk k holds h-feature 256i + 128*(k//4) +
    32*(k%4) + a; the weight packing uses the same row permutation."""
    return hsb[:, 32 * k : 32 * k + 32]


def _build_nc():
    nc = bacc.Bacc("TRN2", target_bir_lowering=False, debug=False, num_devices=NCORES)

    cb = nc.declare_dram_parameter("CB", [128, NB], BF16, isOutput=False)
    cf = nc.declare_dram_parameter("CF", [128, NF], F32, isOutput=False)
    o = nc.declare_dram_parameter("O", [T, 128, YW], F32, isOutput=True)

    with tile.TileContext(nc) as tc:
        with (
            tc.tile_pool(name="wpool", bufs=1) as wpool,
            tc.tile_pool(name="state", bufs=2) as spool,
            tc.tile_pool(name="act", bufs=2) as apool,
            tc.tile_pool(name="gps", bufs=1, space="PSUM") as gpool,
        ):
            CB = wpool.tile([128, NB], BF16, tag="CB")
            CF = wpool.tile([128, NF], F32, tag="CF")
            nc.sync.dma_start(CB[:], cb[:])
            nc.sync.dma_start(CF[:], cf[:])
            WG = CB[:, 0:WG_N]
            WF = CB[:, WG_N : WG_N + WF_N]
            ONES = CB[0:1, WG_N + WF_N : WG_N + WF_N + 32]
            BIAS = CB[0:1, WG_N + WF_N + 32 : WG_N + WF_N + 32 + 4096]
            IB32 = CB[:, WG_N + WF_N + 32 + 4096 : WG_N + WF_N + 32 + 4128]  # 4x I32
            IB128 = CB[:, WG_N + WF_N + 32 + 4128 : NB]  # bf16 I128
            G0 = CF[:, 0 : NGATE * 256]
            H0T = CF[:, NGATE * 256 : NGATE * 256 + 256]
            IDT = CF[:, NGATE * 256 + 256 : NGATE * 256 + 384]
            ONESF = CF[:, NGATE * 256 + 384 : NF]  # all-ones f32 [128,256]

            # PSUM: 8 banks exactly: gA x2 | gZ | gI | tpZ | tpN0 | tpN1 | tpY
            def mk_gA():
                gA = gpool.tile([128, 512], F32, tag="gA", name="gA", bufs=2)
                return gA

            def mk(tag, n, bufs=1):
                # bank-padded (512 f32) so no two PSUM tiles share a bank;
                # hand back a view of the first n cols
                full = gpool.tile([128, 512], F32, tag=tag, name=tag, bufs=bufs)
                return full[:, 0:n]

            def emit_biasA(gA):
                for j in range(4):
                    nc.tensor.matmul(
                        gA[32 * j : 32 * j + 32, :],
                        ONES[:, 0:32],
                        BIAS[:, 512 * j : 512 * j + 512],
                        start=True, stop=False, tile_position=(0, 32 * j),
                    )

            def emit_biasZI(gt, gi):
                for j in range(4):
                    bofs = 1024 * gi + 256 * j
                    nc.tensor.matmul(
                        gt[32 * j : 32 * j + 32, :],
                        ONES[:, 0:32],
                        BIAS[:, bofs : bofs + 256],
                        start=True, stop=False, tile_position=(0, 32 * j),
                    )

            def mk_gates():
                return mk_gA(), mk("gZ", 256, bufs=2), mk("gI", 256, bufs=2)

            def emit_A(hsb, gA):
                # r|hn pair as single N=512 matmuls; even chunks first so
                # the round 0 only waits on the h0 half of the new state.
                for i, k in enumerate(KEVEN + KODD):
                    lhsT = _hslice(hsb, k)
                    for j in range(4):
                        wofs = (k * 4 + j) * 512
                        nc.tensor.matmul(
                            gA[32 * j : 32 * j + 32, :],
                            lhsT,
                            WG[:, wofs : wofs + 512],
                            start=False,
                            stop=(i == KH - 1),
                            tile_position=(0, 32 * j),
                        )

            def emit_ZI(hsb, gt, gi, stop_last):
                for k in range(KH):
                    lhsT = _hslice(hsb, k)
                    for j in range(4):
                        wofs = 16384 + ((gi * KH + k) * 4 + j) * 256
                        nc.tensor.matmul(
                            gt[32 * j : 32 * j + 32, :],
                            lhsT,
                            WG[:, wofs : wofs + 256],
                            start=False,
                            stop=(stop_last and k == KH - 1),
                            tile_position=(0, 32 * j),
                        )

            def emit_inject(gI, rtb):
                # gI += I128^T @ rt in one full-array matmul: the bf16
                # identity is background-loaded during the in-region, so
                # this is a single cheap N=256 stream (and 3 fewer sem
                # increments on the gI group than per-quadrant injects).
                nc.tensor.matmul(
                    gI[:, :],
                    IB128,
                    rtb[:, :],
                    start=False,
                    stop=True,
                )

            def emit_y(hsb_t, tpY, kset):
                for k in kset:
                    lhsT = _hslice(hsb_t, k)
                    for j in range(4):
                        wofs = (k * 4 + j) * YW
                        nc.tensor.matmul(
                            tpY[32 * j : 32 * j + 32, 0:YW],
                            lhsT,
                            WF[:, wofs : wofs + YW],
                            start=(k == 0),
                            stop=(k == KH - 1),
                            tile_position=(0, 32 * j),
                        )

            def chain_zside(zs, tpZ, hsb_prev):
                """p^T = z^T * h^T per half on DVE (early, off the tail)."""
                pts = []
                for hh in range(2):
                    sl = slice(128 * hh, 128 * hh + 128)
                    pT = apool.tile([128, 128], F32, tag=f"pT{hh}", name=f"pT{hh}")
                    nc.vector.tensor_tensor(pT[:], tpZ[:, sl], hsb_prev[:, sl], ALU.mult)
                    pts.append(pT)
                return pts, (tpZ, hsb_prev)

            def chain_nquarter(n_h, pts, pz, hsb2, hh):
                """per 64-col quarter: bt(n) ; q = (z^T-1)*n^T ;
                hsb'[quarter] = pT - q  (DVE) - each quarter unlocks two
                A-rounds of the next step as soon as it lands."""
                tpZ, _ = pz
                tpN = apool.tile([128, 128], BF16, tag=f"tpN{hh}b", name=f"tpN{hh}b")
                for qq in range(2):
                    sln = slice(64 * qq, 64 * qq + 64)
                    glob = slice(128 * hh + 64 * qq, 128 * hh + 64 * qq + 64)
                    nc.vector.transpose(tpN[:, sln], n_h[:, sln])
                    q = apool.tile([128, 64], F32, tag=f"q{hh}{qq}", name=f"q{hh}{qq}")
                    nc.vector.scalar_tensor_tensor(
                        q[:], tpZ[:, glob], 1.0, tpN[:, sln], ALU.subtract, ALU.mult
                    )
                    nc.vector.tensor_tensor(hsb2[:, glob], pts[hh][:, sln], q[:], ALU.subtract)

            # ---- step 0: gates computed host-side (biases included) ----
            rs0 = apool.tile([128, 256], F32, tag="rs")
            nc.scalar.activation(rs0[:], G0[:, 0:256], AF.Sigmoid)
            zs0 = apool.tile([128, 256], BF16, tag="zs")
            nc.scalar.activation(zs0[:], G0[:, 512:768], AF.Sigmoid)
            rt0 = apool.tile([128, 256], F32, tag="rt0")
            nc.vector.tensor_tensor(rt0[:], rs0[:], G0[:, 256:512], ALU.mult)
            ns0 = apool.tile([128, 256], F32, tag="ns0")
            nc.vector.tensor_tensor(ns0[:], rt0[:], G0[:, 768:1024], ALU.add)
            n0 = apool.tile([128, 256], BF16, tag="n0")
            nc.scalar.activation(n0[:], ns0[:], AF.Tanh)
            tpZ = apool.tile([128, 256], BF16, tag="tpZb")
            nc.vector.transpose(tpZ[:, 0:128], zs0[:, 0:128])
            nc.vector.transpose(tpZ[:, 128:256], zs0[:, 128:256])
            pts, pz = chain_zside(zs0, tpZ, H0T)
            hsb = spool.tile([128, 256], BF16, tag="hsb")
            chain_nquarter(n0[:, 0:128], pts, pz, hsb, 0)
            chain_nquarter(n0[:, 128:256], pts, pz, hsb, 1)

            # gates for step 1: bias seeded up front
            pend = mk_gates()
            emit_biasA(pend[0])
            emit_biasZI(pend[1], 2)
            emit_biasZI(pend[2], 3)

            for t in range(T):
                last = t == T - 1
                tpY = mk("tpY", YW)
                if not last:
                    gA, gZ, gI = pend
                    emit_A(hsb, gA)
                    emit_ZI(hsb, gZ, 0, stop_last=True)

                    rs = apool.tile([128, 256], F32, tag="rs")
                    nc.scalar.activation(rs[:], gA[:, 0:256], AF.Sigmoid)
                    zs = apool.tile([128, 256], BF16, tag="zs")
                    nc.scalar.activation(zs[:], gZ[:], AF.Sigmoid)
                    rtb = apool.tile([128, 256], BF16, tag="rtb")
                    nc.vector.tensor_tensor(rtb[:], rs[:], gA[:, 256:512], ALU.mult)

                    emit_ZI(hsb, gI, 1, stop_last=False)
                    emit_inject(gI, rtb)

                    tpZ = apool.tile([128, 256], BF16, tag="tpZb")
                    nc.vector.transpose(tpZ[:, 0:128], zs[:, 0:128])
                    nc.vector.transpose(tpZ[:, 128:256], zs[:, 128:256])

                    n_h0 = apool.tile([128, 128], BF16, tag="n_h0")
                    nc.scalar.activation(n_h0[:], gI[:, 0:128], AF.Tanh)
                    n_h1 = apool.tile([128, 128], BF16, tag="n_h1")
                    nc.scalar.activation(n_h1[:], gI[:, 128:256], AF.Tanh)

                    pts, pz = chain_zside(zs, tpZ, hsb[:])

                    # PE: next-step bias rides the streams, then the whole
                    # y block fills the chain window.
                    if t < T - 2:
                        pend = mk_gates()
                        emit_biasZI(pend[1], 2)
                        emit_biasZI(pend[2], 3)
                    emit_y(hsb, tpY, KEVEN + KODD)
                    if t < T - 2:
                        emit_biasA(pend[0])

                    # DVE FIFO: the whole h0 chain precedes the h1 ops,
                    # so it is not gated by tanh_h1; quarters stream out.
                    hsb2 = spool.tile([128, 256], BF16, tag="hsb")
                    chain_nquarter(n_h0, pts, pz, hsb2, 0)
                    chain_nquarter(n_h1, pts, pz, hsb2, 1)
                    hsb = hsb2
                else:
                    emit_y(hsb, tpY, KEVEN + KODD)

                ys = apool.tile([128, YW], F32, tag="ys")
                nc.scalar.copy(ys[:], tpY[:, 0:YW])
                nc.sync.dma_start(o[t], ys[:])

    nc.compile()
    return nc


def _pack_bat(M):
    """[32, 4*W] -> [128, W]: row 32j+b holds M[b, W*j : W*j+W]."""
    w = M.shape[1] // 4
    return np.ascontiguousarray(
        M.reshape(BC, 4, w).transpose(1, 0, 2).reshape(128, w)
    )


def _prep_shared(w_ih, w_hh, b_ih, b_hh, w_fc, b_fc):
    wihT = w_ih.T.astype(np.float64)  # [768, 3072]
    whhT = w_hh.T.astype(np.float64)  # [1024, 3072]
    wfcT = w_fc.T.astype(np.float64)  # [1024, 768]
    fold = wfcT @ wihT                # [1024, 3072]
    Wr = fold[:, 0:H] + whhT[:, 0:H]
    Wz = fold[:, H : 2 * H] + whhT[:, H : 2 * H]
    Win = fold[:, 2 * H : 3 * H]
    Whn = whhT[:, 2 * H : 3 * H]

    bfold = b_fc.astype(np.float64) @ wihT  # [3072]
    br = bfold[0:H] + b_ih[0:H] + b_hh[0:H]
    bz = bfold[H : 2 * H] + b_ih[H : 2 * H] + b_hh[H : 2 * H]
    bin_ = bfold[2 * H :] + b_ih[2 * H :]
    bhn = b_hh[2 * H :].astype(np.float64)

    # chunk row permutation matching the DVE block-transposed state:
    # chunk c, partition p=32i+a  ->  h-feature 256i + 128*(c//4) + 32*(c%4) + a
    p = np.arange(128)
    cidx = [256 * (p // 32) + 128 * (c // 4) + 32 * (c % 4) + (p % 32) for c in range(KH)]

    blocks = []
    # r|hn interleaved per (k,j) for N=512 pair matmuls
    for k in range(KH):
        for j in range(4):
            blocks.append(Wr[cidx[k]][:, 256 * j : 256 * j + 256])
            blocks.append(Whn[cidx[k]][:, 256 * j : 256 * j + 256])
    # then z, in blocks (N=256)
    for G in (Wz, Win):
        for k in range(KH):
            for j in range(4):
                blocks.append(G[cidx[k]][:, 256 * j : 256 * j + 256])
    WGp = np.concatenate(blocks, axis=1).astype(ml_dtypes.bfloat16)  # [128, 32768]

    yblocks = []
    for k in range(KH):
        for j in range(4):
            yblocks.append(wfcT[cidx[k]][:, YW * j : YW * j + YW])
    WFp = np.concatenate(yblocks, axis=1).astype(ml_dtypes.bfloat16)  # [128, 6144]

    ones_col = np.zeros((128, 32), ml_dtypes.bfloat16)
    ones_col[0, :] = 1
    # bias layout: j-paired [br_j | bhn_j] (4x512) then bz (1024), bin (1024)
    bias_row = np.empty(4096, np.float64)
    for j in range(4):
        bias_row[512 * j : 512 * j + 256] = br[256 * j : 256 * j + 256]
        bias_row[512 * j + 256 : 512 * j + 512] = bhn[256 * j : 256 * j + 256]
    bias_row[2048:3072] = bz
    bias_row[3072:4096] = bin_
    bias_col = np.zeros((128, 4096), ml_dtypes.bfloat16)
    bias_col[0, :] = bias_row.astype(ml_dtypes.bfloat16)

    ib32 = np.tile(np.eye(32), (4, 1)).astype(ml_dtypes.bfloat16)  # [128, 32]
    ib128 = np.eye(128).astype(ml_dtypes.bfloat16)  # [128, 128]

    CBp = np.concatenate([WGp, WFp, ones_col, bias_col, ib32, ib128], axis=1)  # [128, NB]
    assert CBp.shape[1] == NB
    IDT = np.eye(128, dtype=np.float32)
    return CBp, IDT


def _build_in_maps(inputs):
    src = np.asarray(inputs["src"], np.float32)
    hidden = np.asarray(inputs["hidden"], np.float32)
    w_ih = np.asarray(inputs["w_ih"], np.float32)
    w_hh = np.asarray(inputs["w_hh"], np.float32)
    b_ih = np.asarray(inputs["b_ih"], np.float32)
    b_hh = np.asarray(inputs["b_hh"], np.float32)
    w_fc = np.asarray(inputs["w_fc"], np.float32)
    b_fc = np.asarray(inputs["b_fc"], np.float32)

    CBp, IDT = _prep_shared(w_ih, w_hh, b_ih, b_hh, w_fc, b_fc)

    # step-0 gates on host (f64): from x0=src[0], h0=hidden[0]
    x0 = src[0].astype(np.float64)
    h0 = hidden[0].astype(np.float64)
    gi0 = x0 @ w_ih.T.astype(np.float64) + b_ih.astype(np.float64)
    gh0 = h0 @ w_hh.T.astype(np.float64) + b_hh.astype(np.float64)
    g0r = gi0[:, 0:H] + gh0[:, 0:H]
    g0z = gi0[:, H : 2 * H] + gh0[:, H : 2 * H]
    g0in = gi0[:, 2 * H :]
    g0hn = gh0[:, 2 * H :]

    in_maps = []
    for c in range(NCORES):
        sl = slice(BC * c, BC * (c + 1))
        G0 = np.concatenate(
            [
                _pack_bat(g0r[sl]),
                _pack_bat(g0hn[sl]),
                _pack_bat(g0z[sl]),
                _pack_bat(g0in[sl]),
            ],
            axis=1,
        )  # [128, 1024] in region order r|hn|z|in
        HP0 = _pack_bat(h0[sl])  # [128, 256]
        # block-transposed state layout: per half, 32x32 blocks transposed
        # in place (matches nc.vector.transpose semantics)
        def blockT(X):
            return np.ascontiguousarray(
                X.reshape(4, 32, 4, 32).transpose(0, 3, 2, 1).reshape(128, 128)
            )
        H0T = np.concatenate(
            [blockT(HP0[:, 0:128]), blockT(HP0[:, 128:256])], axis=1
        )
        CFp = np.concatenate([G0, H0T, IDT, np.ones((128, 256), np.float32)], axis=1).astype(np.float32)
        assert CFp.shape[1] == NF
        in_maps.append(dict(CB=CBp, CF=CFp))
    return in_maps


def kernel(src, tgt, hidden, w_ih, w_hh, b_ih, b_hh, w_fc, b_fc, **_kw):
    global _COMPILED
    b_fc = np.asarray(b_fc, np.float32)

    if _COMPILED is None:
        _COMPILED = _build_nc()
    nc = _COMPILED

    in_maps = _build_in_maps(
        dict(src=src, hidden=hidden, w_ih=w_ih, w_hh=w_hh, b_ih=b_ih,
             b_hh=b_hh, w_fc=w_fc, b_fc=b_fc)
    )

    res = run_bass_kernel_spmd(nc, in_maps, list(range(NCORES)))

    out = np.empty((T, B, O), np.float32)
    for c in range(NCORES):
        sl = slice(BC * c, BC * (c + 1))
        oc = np.asarray(res.results[c]["O"])  # [T, 128, 192]
        out[:, sl, :] = (
            oc.reshape(T, 4, BC, YW).transpose(0, 2, 1, 3).reshape(T, BC, O)
        )
    out += b_fc[None, None, :]
    return out


# revision 26
# speedup vs baseline: 1.2165x; 1.0036x over previous
"""GRU decoder Trainium2 kernel (data-parallel over batch, 8 cores).

Reference (per step t, PyTorch nn.GRU gate order r,z,n):
    gi = x @ w_ih.T + b_ih ; gh = h @ w_hh.T + b_hh
    r = sig(i_r + h_r); z = sig(i_z + h_z); n = tanh(i_n + r * h_n)
    h' = (1-z)*n + z*h ; y = h' @ w_fc.T + b_fc ; x <- y
Shapes: H=1024, O=768, B=256, T=256.  Each core handles 32 batch rows.

Structure (v20 - v19 + inject slotted mid in-stream):
  * x_t = y_{t-1} folds into the hidden-side matmuls, so every recurrent
    matmul contracts over H=1024: regions r, hn (= h_n), z, in (= i_n).
  * The state lives ONLY as hsb = h'^T (bf16, PE lhsT layout).
  * rt = sig(r)*hn (bf16) is ACCUMULATED INTO the gI PSUM by per-quadrant
    diagonal-identity matmuls, so n = tanh(gI) reads PSUM directly - no
    DVE add / extra sem hop on the critical tail.  bf16 rt keeps the
    inject to one cheap round (fp32 would lower to 2 LOW_HIGH passes).
  * The n-side tail is HALVED: tanh / n^T-transpose / vT / hsb' run per
    128-col half into per-bank PSUM tiles, and the next step's r|hn
    matmuls are issued even-chunks-first (even chunks only read the h0
    half of the fresh state), so they start one half earlier.
  * y_t writes its own PSUM bank (no DVE-read-vs-PE-write bank stall on
    the combine) and is split around the n^T transposes to fill PE gaps.
  * Biases seed PSUM via K=1 ones-row matmuls.  One start=True per bank.
  * Step-0 gates come from the host; b_fc is added on the host.
"""

import numpy as np
import ml_dtypes

import concourse.bass as bass
import concourse.bacc as bacc
import concourse.tile as tile
from concourse import mybir
from concourse.bass_utils import run_bass_kernel_spmd

H = 1024
O = 768
B = 256
T = 256
NCORES = 8
BC = B // NCORES  # 32 batch rows per core

KH = H // 128  # 8 contraction chunks
NGATE = 4      # regions r, hn, z, in (issue order)
YW = O // 4    # 192 y cols per quadrant

F32 = mybir.dt.float32
BF16 = mybir.dt.bfloat16
AF = mybir.ActivationFunctionType
ALU = mybir.AluOpType

_COMPILED = None

# bf16 const layout: WG | WF | ONES | BIAS | IB32 | IB128
WG_N = NGATE * KH * 4 * 256   # 32768
WF_N = KH * 4 * YW            # 6144
NB = WG_N + WF_N + 32 + 4096 + 32 + 128  # 43200
# f32 const layout: G0 (r|hn|z|in) | H0T | IDT | ONESF
NF = NGATE * 256 + 256 + 128 + 256  # 1664

KEVEN = (0, 1, 2, 3)
KODD = (4, 5, 6, 7)


def _hslice(hsb, k):
    """lhsT chunk k from the block-transposed state tile: cols 32k..32k+32.
    Partition 32i+a of chunk k holds h-feature 256i + 128*(k//4) +
    32*(k%4) + a; the weight packing uses the same row permutation."""
    return hsb[:, 32 * k : 32 * k + 32]


def _build_nc():
    nc = bacc.Bacc("TRN2", target_bir_lowering=False, debug=False, num_devices=NCORES)

    cb = nc.declare_dram_parameter("CB", [128, NB], BF16, isOutput=False)
    cf = nc.declare_dram_parameter("CF", [128, NF], F32, isOutput=False)
    o = nc.declare_dram_parameter("O", [T, 128, YW], F32, isOutput=True)

    with tile.TileContext(nc) as tc:
        with (
            tc.tile_pool(name="wpool", bufs=1) as wpool,
            tc.tile_pool(name="state", bufs=2) as spool,
            tc.tile_pool(name="act", bufs=2) as apool,
            tc.tile_pool(name="gps", bufs=1, space="PSUM") as gpool,
        ):
            CB = wpool.tile([128, NB], BF16, tag="CB")
            CF = wpool.tile([128, NF], F32, tag="CF")
            nc.sync.dma_start(CB[:], cb[:])
            nc.sync.dma_start(CF[:], cf[:])
            WG = CB[:, 0:WG_N]
            WF = CB[:, WG_N : WG_N + WF_N]
            ONES = CB[0:1, WG_N + WF_N : WG_N + WF_N + 32]
            BIAS = CB[0:1, WG_N + WF_N + 32 : WG_N + WF_N + 32 + 4096]
            IB32 = CB[:, WG_N + WF_N + 32 + 4096 : WG_N + WF_N + 32 + 4128]  # 4x I32
            IB128 = CB[:, WG_N + WF_N + 32 + 4128 : NB]  # bf16 I128
            G0 = CF[:, 0 : NGATE * 256]
            H0T = CF[:, NGATE * 256 : NGATE * 256 + 256]
            IDT = CF[:, NGATE * 256 + 256 : NGATE * 256 + 384]
            ONESF = CF[:, NGATE * 256 + 384 : NF]  # all-ones f32 [128,256]

            # PSUM: 8 banks exactly: gA x2 | gZ | gI | tpZ | tpN0 | tpN1 | tpY
            def mk_gA():
                gA = gpool.tile([128, 512], F32, tag="gA", name="gA", bufs=2)
                return gA

            def mk(tag, n, bufs=1):
                # bank-padded (512 f32) so no two PSUM tiles share a bank;
                # hand back a view of the first n cols
                full = gpool.tile([128, 512], F32, tag=tag, name=tag, bufs=bufs)
                return full[:, 0:n]

            def emit_biasA(gA):
                for j in range(4):
                    nc.tensor.matmul(
                        gA[32 * j : 32 * j + 32, :],
                        ONES[:, 0:32],
                        BIAS[:, 512 * j : 512 * j + 512],
                        start=True, stop=False, tile_position=(0, 32 * j),
                    )

            def emit_biasZI(gt, gi):
                for j in range(4):
                    bofs = 1024 * gi + 256 * j
                    nc.tensor.matmul(
                        gt[32 * j : 32 * j + 32, :],
                        ONES[:, 0:32],
                        BIAS[:, bofs : bofs + 256],
                        start=True, stop=False, tile_position=(0, 32 * j),
                    )

            def mk_gates():
                return mk_gA(), mk("gZ", 256, bufs=2), mk("gI", 256, bufs=2)

            def emit_A(hsb, gA):
                # r|hn pair as single N=512 matmuls; even chunks first so
                # the round 0 only waits on the h0 half of the new state.
                for i, k in enumerate(KEVEN + KODD):
                    lhsT = _hslice(hsb, k)
                    for j in range(4):
                        wofs = (k * 4 + j) * 512
                        nc.tensor.matmul(
                            gA[32 * j : 32 * j + 32, :],
                            lhsT,
                            WG[:, wofs : wofs + 512],
                            start=False,
                            stop=(i == KH - 1),
                            tile_position=(0, 32 * j),
                        )

            def emit_ZI(hsb, gt, gi, stop_last):
                for k in range(KH):
                    lhsT = _hslice(hsb, k)
                    for j in range(4):
                        wofs = 16384 + ((gi * KH + k) * 4 + j) * 256
                        nc.tensor.matmul(
                            gt[32 * j : 32 * j + 32, :],
                            lhsT,
                            WG[:, wofs : wofs + 256],
                            start=False,
                            stop=(stop_last and k == KH - 1),
                            tile_position=(0, 32 * j),
                        )

            def emit_inject(gI, rtb):
                # gI += I128^T @ rt in one full-array matmul: the bf16
                # identity is background-loaded during the in-region, so
                # this is a single cheap N=256 stream (and 3 fewer sem
                # increments on the gI group than per-quadrant injects).
                nc.tensor.matmul(
                    gI[:, :],
                    IB128,
                    rtb[:, :],
                    start=False,
                    stop=False,
                )

            def emit_y(hsb_t, tpY, kset):
                for k in kset:
                    lhsT = _hslice(hsb_t, k)
                    for j in range(4):
                        wofs = (k * 4 + j) * YW
                        nc.tensor.matmul(
                            tpY[32 * j : 32 * j + 32, 0:YW],
                            lhsT,
                            WF[:, wofs : wofs + YW],
                            start=(k == 0),
                            stop=(k == KH - 1),
                            tile_position=(0, 32 * j),
                        )

            def chain_zside(zs, tpZ, hsb_prev):
                """p^T = z^T * h^T per half on DVE (early, off the tail)."""
                pts = []
                for hh in range(2):
                    sl = slice(128 * hh, 128 * hh + 128)
                    pT = apool.tile([128, 128], F32, tag=f"pT{hh}", name=f"pT{hh}")
                    nc.vector.tensor_tensor(pT[:], tpZ[:, sl], hsb_prev[:, sl], ALU.mult)
                    pts.append(pT)
                return pts, (tpZ, hsb_prev)

            def chain_nquarter(n_h, pts, pz, hsb2, hh):
                """per 64-col quarter: bt(n) ; q = (z^T-1)*n^T ;
                hsb'[quarter] = pT - q  (DVE) - each quarter unlocks two
                A-rounds of the next step as soon as it lands."""
                tpZ, _ = pz
                tpN = apool.tile([128, 128], BF16, tag=f"tpN{hh}b", name=f"tpN{hh}b")
                for qq in range(2):
                    sln = slice(64 * qq, 64 * qq + 64)
                    glob = slice(128 * hh + 64 * qq, 128 * hh + 64 * qq + 64)
                    nc.vector.transpose(tpN[:, sln], n_h[:, sln])
                    q = apool.tile([128, 64], F32, tag=f"q{hh}{qq}", name=f"q{hh}{qq}")
                    nc.vector.scalar_tensor_tensor(
                        q[:], tpZ[:, glob], 1.0, tpN[:, sln], ALU.subtract, ALU.mult
                    )
                    nc.vector.tensor_tensor(hsb2[:, glob], pts[hh][:, sln], q[:], ALU.subtract)

            # ---- step 0: gates computed host-side (biases included) ----
            rs0 = apool.tile([128, 256], F32, tag="rs")
            nc.scalar.activation(rs0[:], G0[:, 0:256], AF.Sigmoid)
            zs0 = apool.tile([128, 256], BF16, tag="zs")
            nc.scalar.activation(zs0[:], G0[:, 512:768], AF.Sigmoid)
            rt0 = apool.tile([128, 256], F32, tag="rt0")
            nc.vector.tensor_tensor(rt0[:], rs0[:], G0[:, 256:512], ALU.mult)
            ns0 = apool.tile([128, 256], F32, tag="ns0")
            nc.vector.tensor_tensor(ns0[:], rt0[:], G0[:, 768:1024], ALU.add)
            n0 = apool.tile([128, 256], BF16, tag="n0")
            nc.scalar.activation(n0[:], ns0[:], AF.Tanh)
            tpZ = apool.tile([128, 256], BF16, tag="tpZb")
            nc.vector.transpose(tpZ[:, 0:128], zs0[:, 0:128])
            nc.vector.transpose(tpZ[:, 128:256], zs0[:, 128:256])
            pts, pz = chain_zside(zs0, tpZ, H0T)
            hsb = spool.tile([128, 256], BF16, tag="hsb")
            chain_nquarter(n0[:, 0:128], pts, pz, hsb, 0)
            chain_nquarter(n0[:, 128:256], pts, pz, hsb, 1)

            # gates for step 1: bias seeded up front
            pend = mk_gates()
            emit_biasA(pend[0])
            emit_biasZI(pend[1], 2)
            emit_biasZI(pend[2], 3)

            for t in range(T):
                last = t == T - 1
                tpY = mk("tpY", YW)
                if not last:
                    gA, gZ, gI = pend
                    emit_A(hsb, gA)
                    emit_ZI(hsb, gZ, 0, stop_last=True)

                    rs = apool.tile([128, 256], F32, tag="rs")
                    nc.scalar.activation(rs[:], gA[:, 0:256], AF.Sigmoid)
                    zs = apool.tile([128, 256], BF16, tag="zs")
                    nc.scalar.activation(zs[:], gZ[:], AF.Sigmoid)
                    rtb = apool.tile([128, 256], BF16, tag="rtb")
                    nc.vector.tensor_tensor(rtb[:], rs[:], gA[:, 256:512], ALU.mult)

                    # in-region with the inject slotted mid-stream (PSUM
                    # accumulation is commutative): gI then finalizes at
                    # the last in-round, ~110ns earlier than inject-last.
                    for k in range(KH):
                        if k == 3:
                            emit_inject(gI, rtb)
                        lhsT = _hslice(hsb, k)
                        for j in range(4):
                            wofs = 16384 + ((KH + k) * 4 + j) * 256
                            nc.tensor.matmul(
                                gI[32 * j : 32 * j + 32, :],
                                lhsT,
                                WG[:, wofs : wofs + 256],
                                start=False,
                                stop=(k == KH - 1),
                                tile_position=(0, 32 * j),
                            )

                    tpZ = apool.tile([128, 256], BF16, tag="tpZb")
                    nc.vector.transpose(tpZ[:, 0:128], zs[:, 0:128])
                    nc.vector.transpose(tpZ[:, 128:256], zs[:, 128:256])

                    n_h0 = apool.tile([128, 128], BF16, tag="n_h0")
                    nc.scalar.activation(n_h0[:], gI[:, 0:128], AF.Tanh)
                    n_h1 = apool.tile([128, 128], BF16, tag="n_h1")
                    nc.scalar.activation(n_h1[:], gI[:, 128:256], AF.Tanh)

                    pts, pz = chain_zside(zs, tpZ, hsb[:])

                    # PE: next-step bias rides the streams, then the whole
                    # y block fills the chain window.
                    if t < T - 2:
                        pend = mk_gates()
                        emit_biasZI(pend[1], 2)
                        emit_biasZI(pend[2], 3)
                    emit_y(hsb, tpY, KEVEN + KODD)
                    if t < T - 2:
                        emit_biasA(pend[0])

                    # DVE FIFO: the whole h0 chain precedes the h1 ops,
                    # so it is not gated by tanh_h1; quarters stream out.
                    hsb2 = spool.tile([128, 256], BF16, tag="hsb")
                    chain_nquarter(n_h0, pts, pz, hsb2, 0)
                    chain_nquarter(n_h1, pts, pz, hsb2, 1)
                    hsb = hsb2
                else:
                    emit_y(hsb, tpY, KEVEN + KODD)

                ys = apool.tile([128, YW], F32, tag="ys")
                nc.scalar.copy(ys[:], tpY[:, 0:YW])
                nc.sync.dma_start(o[t], ys[:])

    nc.compile()
    return nc


def _pack_bat(M):
    """[32, 4*W] -> [128, W]: row 32j+b holds M[b, W*j : W*j+W]."""
    w = M.shape[1] // 4
    return np.ascontiguousarray(
        M.reshape(BC, 4, w).transpose(1, 0, 2).reshape(128, w)
    )


def _prep_shared(w_ih, w_hh, b_ih, b_hh, w_fc, b_fc):
    wihT = w_ih.T.astype(np.float64)  # [768, 3072]
    whhT = w_hh.T.astype(np.float64)  # [1024, 3072]
    wfcT = w_fc.T.astype(np.float64)  # [1024, 768]
    fold = wfcT @ wihT                # [1024, 3072]
    Wr = fold[:, 0:H] + whhT[:, 0:H]
    Wz = fold[:, H : 2 * H] + whhT[:, H : 2 * H]
    Win = fold[:, 2 * H : 3 * H]
    Whn = whhT[:, 2 * H : 3 * H]

    bfold = b_fc.astype(np.float64) @ wihT  # [3072]
    br = bfold[0:H] + b_ih[0:H] + b_hh[0:H]
    bz = bfold[H : 2 * H] + b_ih[H : 2 * H] + b_hh[H : 2 * H]
    bin_ = bfold[2 * H :] + b_ih[2 * H :]
    bhn = b_hh[2 * H :].astype(np.float64)

    # chunk row permutation matching the DVE block-transposed state:
    # chunk c, partition p=32i+a  ->  h-feature 256i + 128*(c//4) + 32*(c%4) + a
    p = np.arange(128)
    cidx = [256 * (p // 32) + 128 * (c // 4) + 32 * (c % 4) + (p % 32) for c in range(KH)]

    blocks = []
    # r|hn interleaved per (k,j) for N=512 pair matmuls
    for k in range(KH):
        for j in range(4):
            blocks.append(Wr[cidx[k]][:, 256 * j : 256 * j + 256])
            blocks.append(Whn[cidx[k]][:, 256 * j : 256 * j + 256])
    # then z, in blocks (N=256)
    for G in (Wz, Win):
        for k in range(KH):
            for j in range(4):
                blocks.append(G[cidx[k]][:, 256 * j : 256 * j + 256])
    WGp = np.concatenate(blocks, axis=1).astype(ml_dtypes.bfloat16)  # [128, 32768]

    yblocks = []
    for k in range(KH):
        for j in range(4):
            yblocks.append(wfcT[cidx[k]][:, YW * j : YW * j + YW])
    WFp = np.concatenate(yblocks, axis=1).astype(ml_dtypes.bfloat16)  # [128, 6144]

    ones_col = np.zeros((128, 32), ml_dtypes.bfloat16)
    ones_col[0, :] = 1
    # bias layout: j-paired [br_j | bhn_j] (4x512) then bz (1024), bin (1024)
    bias_row = np.empty(4096, np.float64)
    for j in range(4):
        bias_row[512 * j : 512 * j + 256] = br[256 * j : 256 * j + 256]
        bias_row[512 * j + 256 : 512 * j + 512] = bhn[256 * j : 256 * j + 256]
    bias_row[2048:3072] = bz
    bias_row[3072:4096] = bin_
    bias_col = np.zeros((128, 4096), ml_dtypes.bfloat16)
    bias_col[0, :] = bias_row.astype(ml_dtypes.bfloat16)

    ib32 = np.tile(np.eye(32), (4, 1)).astype(ml_dtypes.bfloat16)  # [128, 32]
    ib128 = np.eye(128).astype(ml_dtypes.bfloat16)  # [128, 128]

    CBp = np.concatenate([WGp, WFp, ones_col, bias_col, ib32, ib128], axis=1)  # [128, NB]
    assert CBp.shape[1] == NB
    IDT = np.eye(128, dtype=np.float32)
    return CBp, IDT


def _build_in_maps(inputs):
    src = np.asarray(inputs["src"], np.float32)
    hidden = np.asarray(inputs["hidden"], np.float32)
    w_ih = np.asarray(inputs["w_ih"], np.float32)
    w_hh = np.asarray(inputs["w_hh"], np.float32)
    b_ih = np.asarray(inputs["b_ih"], np.float32)
    b_hh = np.asarray(inputs["b_hh"], np.float32)
    w_fc = np.asarray(inputs["w_fc"], np.float32)
    b_fc = np.asarray(inputs["b_fc"], np.float32)

    CBp, IDT = _prep_shared(w_ih, w_hh, b_ih, b_hh, w_fc, b_fc)

    # step-0 gates on host (f64): from x0=src[0], h0=hidden[0]
    x0 = src[0].astype(np.float64)
    h0 = hidden[0].astype(np.float64)
    gi0 = x0 @ w_ih.T.astype(np.float64) + b_ih.astype(np.float64)
    gh0 = h0 @ w_hh.T.astype(np.float64) + b_hh.astype(np.float64)
    g0r = gi0[:, 0:H] + gh0[:, 0:H]
    g0z = gi0[:, H : 2 * H] + gh0[:, H : 2 * H]
    g0in = gi0[:, 2 * H :]
    g0hn = gh0[:, 2 * H :]

    in_maps = []
    for c in range(NCORES):
        sl = slice(BC * c, BC * (c + 1))
        G0 = np.concatenate(
            [
                _pack_bat(g0r[sl]),
                _pack_bat(g0hn[sl]),
                _pack_bat(g0z[sl]),
                _pack_bat(g0in[sl]),
            ],
            axis=1,
        )  # [128, 1024] in region order r|hn|z|in
        HP0 = _pack_bat(h0[sl])  # [128, 256]
        # block-transposed state layout: per half, 32x32 blocks transposed
        # in place (matches nc.vector.transpose semantics)
        def blockT(X):
            return np.ascontiguousarray(
                X.reshape(4, 32, 4, 32).transpose(0, 3, 2, 1).reshape(128, 128)
            )
        H0T = np.concatenate(
            [blockT(HP0[:, 0:128]), blockT(HP0[:, 128:256])], axis=1
        )
        CFp = np.concatenate([G0, H0T, IDT, np.ones((128, 256), np.float32)], axis=1).astype(np.float32)
        assert CFp.shape[1] == NF
        in_maps.append(dict(CB=CBp, CF=CFp))
    return in_maps


def kernel(src, tgt, hidden, w_ih, w_hh, b_ih, b_hh, w_fc, b_fc, **_kw):
    global _COMPILED
    b_fc = np.asarray(b_fc, np.float32)

    if _COMPILED is None:
        _COMPILED = _build_nc()
    nc = _COMPILED

    in_maps = _build_in_maps(
        dict(src=src, hidden=hidden, w_ih=w_ih, w_hh=w_hh, b_ih=b_ih,
             b_hh=b_hh, w_fc=w_fc, b_fc=b_fc)
    )

    res = run_bass_kernel_spmd(nc, in_maps, list(range(NCORES)))

    out = np.empty((T, B, O), np.float32)
    for c in range(NCORES):
        sl = slice(BC * c, BC * (c + 1))
        oc = np.asarray(res.results[c]["O"])  # [T, 128, 192]
        out[:, sl, :] = (
            oc.reshape(T, 4, BC, YW).transpose(0, 2, 1, 3).reshape(T, BC, O)
        )
    out += b_fc[None, None, :]
    return out


# revision 27
# speedup vs baseline: 1.2220x; 1.0045x over previous
"""GRU decoder Trainium2 kernel (data-parallel over batch, 8 cores).

Reference (per step t, PyTorch nn.GRU gate order r,z,n):
    gi = x @ w_ih.T + b_ih ; gh = h @ w_hh.T + b_hh
    r = sig(i_r + h_r); z = sig(i_z + h_z); n = tanh(i_n + r * h_n)
    h' = (1-z)*n + z*h ; y = h' @ w_fc.T + b_fc ; x <- y
Shapes: H=1024, O=768, B=256, T=256.  Each core handles 32 batch rows.

Structure (v21 - v20 + consumption-ordered constant DMAs):
  * x_t = y_{t-1} folds into the hidden-side matmuls, so every recurrent
    matmul contracts over H=1024: regions r, hn (= h_n), z, in (= i_n).
  * The state lives ONLY as hsb = h'^T (bf16, PE lhsT layout).
  * rt = sig(r)*hn (bf16) is ACCUMULATED INTO the gI PSUM by per-quadrant
    diagonal-identity matmuls, so n = tanh(gI) reads PSUM directly - no
    DVE add / extra sem hop on the critical tail.  bf16 rt keeps the
    inject to one cheap round (fp32 would lower to 2 LOW_HIGH passes).
  * The n-side tail is HALVED: tanh / n^T-transpose / vT / hsb' run per
    128-col half into per-bank PSUM tiles, and the next step's r|hn
    matmuls are issued even-chunks-first (even chunks only read the h0
    half of the fresh state), so they start one half earlier.
  * y_t writes its own PSUM bank (no DVE-read-vs-PE-write bank stall on
    the combine) and is split around the n^T transposes to fill PE gaps.
  * Biases seed PSUM via K=1 ones-row matmuls.  One start=True per bank.
  * Step-0 gates come from the host; b_fc is added on the host.
"""

import numpy as np
import ml_dtypes

import concourse.bass as bass
import concourse.bacc as bacc
import concourse.tile as tile
from concourse import mybir
from concourse.bass_utils import run_bass_kernel_spmd

H = 1024
O = 768
B = 256
T = 256
NCORES = 8
BC = B // NCORES  # 32 batch rows per core

KH = H // 128  # 8 contraction chunks
NGATE = 4      # regions r, hn, z, in (issue order)
YW = O // 4    # 192 y cols per quadrant

F32 = mybir.dt.float32
BF16 = mybir.dt.bfloat16
AF = mybir.ActivationFunctionType
ALU = mybir.AluOpType

_COMPILED = None

# bf16 const layout: WG | WF | ONES | BIAS | IB32 | IB128
WG_N = NGATE * KH * 4 * 256   # 32768
WF_N = KH * 4 * YW            # 6144
NB = WG_N + WF_N + 32 + 4096 + 32 + 128  # 43200
# f32 const layout: G0 (r|hn|z|in) | H0T | IDT | ONESF
NF = NGATE * 256 + 256 + 128 + 256  # 1664

KEVEN = (0, 1, 2, 3)
KODD = (4, 5, 6, 7)


def _hslice(hsb, k):
    """lhsT chunk k from the block-transposed state tile: cols 32k..32k+32.
    Partition 32i+a of chunk k holds h-feature 256i + 128*(k//4) +
    32*(k%4) + a; the weight packing uses the same row permutation."""
    return hsb[:, 32 * k : 32 * k + 32]


def _build_nc():
    nc = bacc.Bacc("TRN2", target_bir_lowering=False, debug=False, num_devices=NCORES)

    cb = nc.declare_dram_parameter("CB", [128, NB], BF16, isOutput=False)
    cf = nc.declare_dram_parameter("CF", [128, NF], F32, isOutput=False)
    o = nc.declare_dram_parameter("O", [T, 128, YW], F32, isOutput=True)

    with tile.TileContext(nc) as tc:
        with (
            tc.tile_pool(name="wpool", bufs=1) as wpool,
            tc.tile_pool(name="state", bufs=2) as spool,
            tc.tile_pool(name="act", bufs=2) as apool,
            tc.tile_pool(name="gps", bufs=1, space="PSUM") as gpool,
        ):
            CB = wpool.tile([128, NB], BF16, tag="CB")
            CF = wpool.tile([128, NF], F32, tag="CF")
            # constants arrive in consumption order so the pipeline fills
            # while the bulk weights are still in flight (one 10.5MB DMA
            # stalled the first matmul ~36us)
            nc.sync.dma_start(CF[:], cf[:])                     # step-0 gates
            nc.sync.dma_start(CB[:, 38912:NB], cb[:, 38912:NB])  # ONES|BIAS|IB*
            nc.sync.dma_start(CB[:, 0:16384], cb[:, 0:16384])    # WG: r|hn
            nc.sync.dma_start(CB[:, 16384:24576], cb[:, 16384:24576])  # WG: z
            nc.sync.dma_start(CB[:, 24576:32768], cb[:, 24576:32768])  # WG: in
            nc.sync.dma_start(CB[:, 32768:38912], cb[:, 32768:38912])  # WF
            WG = CB[:, 0:WG_N]
            WF = CB[:, WG_N : WG_N + WF_N]
            ONES = CB[0:1, WG_N + WF_N : WG_N + WF_N + 32]
            BIAS = CB[0:1, WG_N + WF_N + 32 : WG_N + WF_N + 32 + 4096]
            IB32 = CB[:, WG_N + WF_N + 32 + 4096 : WG_N + WF_N + 32 + 4128]  # 4x I32
            IB128 = CB[:, WG_N + WF_N + 32 + 4128 : NB]  # bf16 I128
            G0 = CF[:, 0 : NGATE * 256]
            H0T = CF[:, NGATE * 256 : NGATE * 256 + 256]
            IDT = CF[:, NGATE * 256 + 256 : NGATE * 256 + 384]
            ONESF = CF[:, NGATE * 256 + 384 : NF]  # all-ones f32 [128,256]

            # PSUM: 8 banks exactly: gA x2 | gZ | gI | tpZ | tpN0 | tpN1 | tpY
            def mk_gA():
                gA = gpool.tile([128, 512], F32, tag="gA", name="gA", bufs=2)
                return gA

            def mk(tag, n, bufs=1):
                # bank-padded (512 f32) so no two PSUM tiles share a bank;
                # hand back a view of the first n cols
                full = gpool.tile([128, 512], F32, tag=tag, name=tag, bufs=bufs)
                return full[:, 0:n]

            def emit_biasA(gA):
                for j in range(4):
                    nc.tensor.matmul(
                        gA[32 * j : 32 * j + 32, :],
                        ONES[:, 0:32],
                        BIAS[:, 512 * j : 512 * j + 512],
                        start=True, stop=False, tile_position=(0, 32 * j),
                    )

            def emit_biasZI(gt, gi):
                for j in range(4):
                    bofs = 1024 * gi + 256 * j
                    nc.tensor.matmul(
                        gt[32 * j : 32 * j + 32, :],
                        ONES[:, 0:32],
                        BIAS[:, bofs : bofs + 256],
                        start=True, stop=False, tile_position=(0, 32 * j),
                    )

            def mk_gates():
                return mk_gA(), mk("gZ", 256, bufs=2), mk("gI", 256, bufs=2)

            def emit_A(hsb, gA):
                # r|hn pair as single N=512 matmuls; even chunks first so
                # the round 0 only waits on the h0 half of the new state.
                for i, k in enumerate(KEVEN + KODD):
                    lhsT = _hslice(hsb, k)
                    for j in range(4):
                        wofs = (k * 4 + j) * 512
                        nc.tensor.matmul(
                            gA[32 * j : 32 * j + 32, :],
                            lhsT,
                            WG[:, wofs : wofs + 512],
                            start=False,
                            stop=(i == KH - 1),
                            tile_position=(0, 32 * j),
                        )

            def emit_ZI(hsb, gt, gi, stop_last):
                for k in range(KH):
                    lhsT = _hslice(hsb, k)
                    for j in range(4):
                        wofs = 16384 + ((gi * KH + k) * 4 + j) * 256
                        nc.tensor.matmul(
                            gt[32 * j : 32 * j + 32, :],
                            lhsT,
                            WG[:, wofs : wofs + 256],
                            start=False,
                            stop=(stop_last and k == KH - 1),
                            tile_position=(0, 32 * j),
                        )

            def emit_inject(gI, rtb):
                # gI += I128^T @ rt in one full-array matmul: the bf16
                # identity is background-loaded during the in-region, so
                # this is a single cheap N=256 stream (and 3 fewer sem
                # increments on the gI group than per-quadrant injects).
                nc.tensor.matmul(
                    gI[:, :],
                    IB128,
                    rtb[:, :],
                    start=False,
                    stop=False,
                )

            def emit_y(hsb_t, tpY, kset):
                for k in kset:
                    lhsT = _hslice(hsb_t, k)
                    for j in range(4):
                        wofs = (k * 4 + j) * YW
                        nc.tensor.matmul(
                            tpY[32 * j : 32 * j + 32, 0:YW],
                            lhsT,
                            WF[:, wofs : wofs + YW],
                            start=(k == 0),
                            stop=(k == KH - 1),
                            tile_position=(0, 32 * j),
                        )

            def chain_zside(zs, tpZ, hsb_prev):
                """p^T = z^T * h^T per half on DVE (early, off the tail)."""
                pts = []
                for hh in range(2):
                    sl = slice(128 * hh, 128 * hh + 128)
                    pT = apool.tile([128, 128], F32, tag=f"pT{hh}", name=f"pT{hh}")
                    nc.vector.tensor_tensor(pT[:], tpZ[:, sl], hsb_prev[:, sl], ALU.mult)
                    pts.append(pT)
                return pts, (tpZ, hsb_prev)

            def chain_nquarter(n_h, pts, pz, hsb2, hh):
                """per 64-col quarter: bt(n) ; q = (z^T-1)*n^T ;
                hsb'[quarter] = pT - q  (DVE) - each quarter unlocks two
                A-rounds of the next step as soon as it lands."""
                tpZ, _ = pz
                tpN = apool.tile([128, 128], BF16, tag=f"tpN{hh}b", name=f"tpN{hh}b")
                for qq in range(2):
                    sln = slice(64 * qq, 64 * qq + 64)
                    glob = slice(128 * hh + 64 * qq, 128 * hh + 64 * qq + 64)
                    nc.vector.transpose(tpN[:, sln], n_h[:, sln])
                    q = apool.tile([128, 64], F32, tag=f"q{hh}{qq}", name=f"q{hh}{qq}")
                    nc.vector.scalar_tensor_tensor(
                        q[:], tpZ[:, glob], 1.0, tpN[:, sln], ALU.subtract, ALU.mult
                    )
                    nc.vector.tensor_tensor(hsb2[:, glob], pts[hh][:, sln], q[:], ALU.subtract)

            # ---- step 0: gates computed host-side (biases included) ----
            rs0 = apool.tile([128, 256], F32, tag="rs")
            nc.scalar.activation(rs0[:], G0[:, 0:256], AF.Sigmoid)
            zs0 = apool.tile([128, 256], BF16, tag="zs")
            nc.scalar.activation(zs0[:], G0[:, 512:768], AF.Sigmoid)
            rt0 = apool.tile([128, 256], F32, tag="rt0")
            nc.vector.tensor_tensor(rt0[:], rs0[:], G0[:, 256:512], ALU.mult)
            ns0 = apool.tile([128, 256], F32, tag="ns0")
            nc.vector.tensor_tensor(ns0[:], rt0[:], G0[:, 768:1024], ALU.add)
            n0 = apool.tile([128, 256], BF16, tag="n0")
            nc.scalar.activation(n0[:], ns0[:], AF.Tanh)
            tpZ = apool.tile([128, 256], BF16, tag="tpZb")
            nc.vector.transpose(tpZ[:, 0:128], zs0[:, 0:128])
            nc.vector.transpose(tpZ[:, 128:256], zs0[:, 128:256])
            pts, pz = chain_zside(zs0, tpZ, H0T)
            hsb = spool.tile([128, 256], BF16, tag="hsb")
            chain_nquarter(n0[:, 0:128], pts, pz, hsb, 0)
            chain_nquarter(n0[:, 128:256], pts, pz, hsb, 1)

            # gates for step 1: bias seeded up front
            pend = mk_gates()
            emit_biasA(pend[0])
            emit_biasZI(pend[1], 2)
            emit_biasZI(pend[2], 3)

            for t in range(T):
                last = t == T - 1
                tpY = mk("tpY", YW)
                if not last:
                    gA, gZ, gI = pend
                    emit_A(hsb, gA)
                    emit_ZI(hsb, gZ, 0, stop_last=True)

                    rs = apool.tile([128, 256], F32, tag="rs")
                    nc.scalar.activation(rs[:], gA[:, 0:256], AF.Sigmoid)
                    zs = apool.tile([128, 256], BF16, tag="zs")
                    nc.scalar.activation(zs[:], gZ[:], AF.Sigmoid)
                    rtb = apool.tile([128, 256], BF16, tag="rtb")
                    nc.vector.tensor_tensor(rtb[:], rs[:], gA[:, 256:512], ALU.mult)

                    # in-region with the inject slotted mid-stream (PSUM
                    # accumulation is commutative): gI then finalizes at
                    # the last in-round, ~110ns earlier than inject-last.
                    for k in range(KH):
                        if k == 3:
                            emit_inject(gI, rtb)
                        lhsT = _hslice(hsb, k)
                        for j in range(4):
                            wofs = 16384 + ((KH + k) * 4 + j) * 256
                            nc.tensor.matmul(
                                gI[32 * j : 32 * j + 32, :],
                                lhsT,
                                WG[:, wofs : wofs + 256],
                                start=False,
                                stop=(k == KH - 1),
                                tile_position=(0, 32 * j),
                            )

                    tpZ = apool.tile([128, 256], BF16, tag="tpZb")
                    nc.vector.transpose(tpZ[:, 0:128], zs[:, 0:128])
                    nc.vector.transpose(tpZ[:, 128:256], zs[:, 128:256])

                    n_h0 = apool.tile([128, 128], BF16, tag="n_h0")
                    nc.scalar.activation(n_h0[:], gI[:, 0:128], AF.Tanh)
                    n_h1 = apool.tile([128, 128], BF16, tag="n_h1")
                    nc.scalar.activation(n_h1[:], gI[:, 128:256], AF.Tanh)

                    pts, pz = chain_zside(zs, tpZ, hsb[:])

                    # PE: next-step bias rides the streams, then the whole
                    # y block fills the chain window.
                    if t < T - 2:
                        pend = mk_gates()
                        emit_biasZI(pend[1], 2)
                        emit_biasZI(pend[2], 3)
                    emit_y(hsb, tpY, KEVEN + KODD)
                    if t < T - 2:
                        emit_biasA(pend[0])

                    # DVE FIFO: the whole h0 chain precedes the h1 ops,
                    # so it is not gated by tanh_h1; quarters stream out.
                    hsb2 = spool.tile([128, 256], BF16, tag="hsb")
                    chain_nquarter(n_h0, pts, pz, hsb2, 0)
                    chain_nquarter(n_h1, pts, pz, hsb2, 1)
                    hsb = hsb2
                else:
                    emit_y(hsb, tpY, KEVEN + KODD)

                ys = apool.tile([128, YW], F32, tag="ys")
                nc.scalar.copy(ys[:], tpY[:, 0:YW])
                nc.sync.dma_start(o[t], ys[:])

    nc.compile()
    return nc


def _pack_bat(M):
    """[32, 4*W] -> [128, W]: row 32j+b holds M[b, W*j : W*j+W]."""
    w = M.shape[1] // 4
    return np.ascontiguousarray(
        M.reshape(BC, 4, w).transpose(1, 0, 2).reshape(128, w)
    )


def _prep_shared(w_ih, w_hh, b_ih, b_hh, w_fc, b_fc):
    wihT = w_ih.T.astype(np.float64)  # [768, 3072]
    whhT = w_hh.T.astype(np.float64)  # [1024, 3072]
    wfcT = w_fc.T.astype(np.float64)  # [1024, 768]
    fold = wfcT @ wihT                # [1024, 3072]
    Wr = fold[:, 0:H] + whhT[:, 0:H]
    Wz = fold[:, H : 2 * H] + whhT[:, H : 2 * H]
    Win = fold[:, 2 * H : 3 * H]
    Whn = whhT[:, 2 * H : 3 * H]

    bfold = b_fc.astype(np.float64) @ wihT  # [3072]
    br = bfold[0:H] + b_ih[0:H] + b_hh[0:H]
    bz = bfold[H : 2 * H] + b_ih[H : 2 * H] + b_hh[H : 2 * H]
    bin_ = bfold[2 * H :] + b_ih[2 * H :]
    bhn = b_hh[2 * H :].astype(np.float64)

    # chunk row permutation matching the DVE block-transposed state:
    # chunk c, partition p=32i+a  ->  h-feature 256i + 128*(c//4) + 32*(c%4) + a
    p = np.arange(128)
    cidx = [256 * (p // 32) + 128 * (c // 4) + 32 * (c % 4) + (p % 32) for c in range(KH)]

    blocks = []
    # r|hn interleaved per (k,j) for N=512 pair matmuls
    for k in range(KH):
        for j in range(4):
            blocks.append(Wr[cidx[k]][:, 256 * j : 256 * j + 256])
            blocks.append(Whn[cidx[k]][:, 256 * j : 256 * j + 256])
    # then z, in blocks (N=256)
    for G in (Wz, Win):
        for k in range(KH):
            for j in range(4):
                blocks.append(G[cidx[k]][:, 256 * j : 256 * j + 256])
    WGp = np.concatenate(blocks, axis=1).astype(ml_dtypes.bfloat16)  # [128, 32768]

    yblocks = []
    for k in range(KH):
        for j in range(4):
            yblocks.append(wfcT[cidx[k]][:, YW * j : YW * j + YW])
    WFp = np.concatenate(yblocks, axis=1).astype(ml_dtypes.bfloat16)  # [128, 6144]

    ones_col = np.zeros((128, 32), ml_dtypes.bfloat16)
    ones_col[0, :] = 1
    # bias layout: j-paired [br_j | bhn_j] (4x512) then bz (1024), bin (1024)
    bias_row = np.empty(4096, np.float64)
    for j in range(4):
        bias_row[512 * j : 512 * j + 256] = br[256 * j : 256 * j + 256]
        bias_row[512 * j + 256 : 512 * j + 512] = bhn[256 * j : 256 * j + 256]
    bias_row[2048:3072] = bz
    bias_row[3072:4096] = bin_
    bias_col = np.zeros((128, 4096), ml_dtypes.bfloat16)
    bias_col[0, :] = bias_row.astype(ml_dtypes.bfloat16)

    ib32 = np.tile(np.eye(32), (4, 1)).astype(ml_dtypes.bfloat16)  # [128, 32]
    ib128 = np.eye(128).astype(ml_dtypes.bfloat16)  # [128, 128]

    CBp = np.concatenate([WGp, WFp, ones_col, bias_col, ib32, ib128], axis=1)  # [128, NB]
    assert CBp.shape[1] == NB
    IDT = np.eye(128, dtype=np.float32)
    return CBp, IDT


def _build_in_maps(inputs):
    src = np.asarray(inputs["src"], np.float32)
    hidden = np.asarray(inputs["hidden"], np.float32)
    w_ih = np.asarray(inputs["w_ih"], np.float32)
    w_hh = np.asarray(inputs["w_hh"], np.float32)
    b_ih = np.asarray(inputs["b_ih"], np.float32)
    b_hh = np.asarray(inputs["b_hh"], np.float32)
    w_fc = np.asarray(inputs["w_fc"], np.float32)
    b_fc = np.asarray(inputs["b_fc"], np.float32)

    CBp, IDT = _prep_shared(w_ih, w_hh, b_ih, b_hh, w_fc, b_fc)

    # step-0 gates on host (f64): from x0=src[0], h0=hidden[0]
    x0 = src[0].astype(np.float64)
    h0 = hidden[0].astype(np.float64)
    gi0 = x0 @ w_ih.T.astype(np.float64) + b_ih.astype(np.float64)
    gh0 = h0 @ w_hh.T.astype(np.float64) + b_hh.astype(np.float64)
    g0r = gi0[:, 0:H] + gh0[:, 0:H]
    g0z = gi0[:, H : 2 * H] + gh0[:, H : 2 * H]
    g0in = gi0[:, 2 * H :]
    g0hn = gh0[:, 2 * H :]

    in_maps = []
    for c in range(NCORES):
        sl = slice(BC * c, BC * (c + 1))
        G0 = np.concatenate(
            [
                _pack_bat(g0r[sl]),
                _pack_bat(g0hn[sl]),
                _pack_bat(g0z[sl]),
                _pack_bat(g0in[sl]),
            ],
            axis=1,
        )  # [128, 1024] in region order r|hn|z|in
        HP0 = _pack_bat(h0[sl])  # [128, 256]
        # block-transposed state layout: per half, 32x32 blocks transposed
        # in place (matches nc.vector.transpose semantics)
        def blockT(X):
            return np.ascontiguousarray(
                X.reshape(4, 32, 4, 32).transpose(0, 3, 2, 1).reshape(128, 128)
            )
        H0T = np.concatenate(
            [blockT(HP0[:, 0:128]), blockT(HP0[:, 128:256])], axis=1
        )
        CFp = np.concatenate([G0, H0T, IDT, np.ones((128, 256), np.float32)], axis=1).astype(np.float32)
        assert CFp.shape[1] == NF
        in_maps.append(dict(CB=CBp, CF=CFp))
    return in_maps


def kernel(src, tgt, hidden, w_ih, w_hh, b_ih, b_hh, w_fc, b_fc, **_kw):
    global _COMPILED
    b_fc = np.asarray(b_fc, np.float32)

    if _COMPILED is None:
        _COMPILED = _build_nc()
    nc = _COMPILED

    in_maps = _build_in_maps(
        dict(src=src, hidden=hidden, w_ih=w_ih, w_hh=w_hh, b_ih=b_ih,
             b_hh=b_hh, w_fc=w_fc, b_fc=b_fc)
    )

    res = run_bass_kernel_spmd(nc, in_maps, list(range(NCORES)))

    out = np.empty((T, B, O), np.float32)
    for c in range(NCORES):
        sl = slice(BC * c, BC * (c + 1))
        oc = np.asarray(res.results[c]["O"])  # [T, 128, 192]
        out[:, sl, :] = (
            oc.reshape(T, 4, BC, YW).transpose(0, 2, 1, 3).reshape(T, BC, O)
        )
    out += b_fc[None, None, :]
    return out
